# revision 14
# baseline (speedup 1.0000x reference)
"""Pairwise cosine-similarity adjacency exp(-0.5 * cos_sim) on 8 trn2 cores.

Input : x [4, 4096, 512] fp32
Output: exp(-0.5 * (xn @ xn.T)) per batch -> [4, 4096, 4096] fp32,
        xn = x / max(||x||_row, 1e-8)

Sharding: batch b = core // 2, half h = core % 2. Cyclic symmetric cover in
128-row blocks: block-row r computes block-cols (r..r+16) mod 32, so every
unordered block pair is covered; the host mirrors the transposed copies.
Each core handles 16 block-rows (h=1 rows arrive pre-rotated by 2048 so the
device program is SPMD-identical).

Host preps the operand: row-normalize, transpose to [D, N], rotate, scale by
8, cast fp8 e4m3, and interleave K-chunk pairs for DoubleRow matmuls.

Device (raw bass, hand-scheduled semaphores — no TileContext):
  SP   : 8 input DMAs (fine head split so PE starts early), 17 output DMAs
  PE   : 6 warmup matmuls (clock ramp during input DMA), then per block-row
         10 fp8 DoubleRow matmuls (K=256 each) into a 3x[128,1024] +
         2x[128,128] PSUM ring
  ACT  : dummy exp (preloads table), then exps slices A + N per row
  DVE  : raw bf16 copy of slice B per row (host applies exp to those cols)
Host upcasts, exps the DVE columns, and mirrors the strips into the full
[4, 4096, 4096] output.
"""
import sys

sys.path.insert(0, '/opt/trn_rl_repo')

import numpy as np
import ml_dtypes

B, N, D = 4, 4096, 512
N_CORES = 8
NB = N // 128        # 32 block-rows per batch
T = NB // 2 + 1      # 17 block-cols per block-row (cyclic cover)
W = T * 128          # 2176 strip width
ROWS = NB // 2       # 16 block-rows per core
EPS = 1e-8
FP8_SCALE = 8.0
ESCALE = -0.5 / (FP8_SCALE * FP8_SCALE)
SPLITS = (0, 1088, 1536, 2304, 3200, N)   # input DMA column splits

_compiled = {}


def _build():
    import concourse.mybir as mybir
    from concourse import bacc

    fp32 = mybir.dt.float32
    bf16 = mybir.dt.bfloat16
    fp8 = mybir.dt.float8e4
    DR = mybir.MatmulPerfMode.DoubleRow
    Exp = mybir.ActivationFunctionType.Exp

    nc = bacc.Bacc(trn_type="TRN2", target_bir_lowering=False, debug=False,
                   num_devices=N_CORES)
    xa = nc.dram_tensor("xa", [128, 2, N], fp8, kind="ExternalInput")
    xb = nc.dram_tensor("xb", [128, 2, N], fp8, kind="ExternalInput")
    out = nc.dram_tensor("out", [ROWS * 128, W], bf16, kind="ExternalOutput")

    xsa = nc.alloc_sbuf_tensor("xsa", [128, 2, N], fp8)
    xsb = nc.alloc_sbuf_tensor("xsb", [128, 2, N], fp8)
    ots = [nc.alloc_sbuf_tensor(f"ot{j}", [128, W], bf16) for j in range(3)]
    dummy = nc.alloc_sbuf_tensor("warm_act", [128, 1], fp32)
    accs = [nc.alloc_psum_tensor(f"acc{j}", [128, 1024], fp32)
            for j in range(3)]
    naccs = [nc.alloc_psum_tensor(f"nacc{j}", [128, 128], fp32)
             for j in range(2)]

    # One sem per input DMA: a DMA's +16 completion arrives as one increment
    # per DMA engine, and different DMAs' engine completions interleave, so a
    # shared counting sem can hit a threshold before the earlier DMA is done.
    n_in = 2 * (len(SPLITS) - 1)
    s_in = [nc.alloc_semaphore(f"s_in{k}") for k in range(n_in)]
    s_mm = nc.alloc_semaphore("s_mm")
    s_act = nc.alloc_semaphore("s_act")
    s_dve = nc.alloc_semaphore("s_dve")
    # Per-ot-slot output sems (safe: the next DMA on a slot can't issue until
    # the exp that waits on the previous one has run).
    s_out = [nc.alloc_semaphore(f"s_out{j}") for j in range(3)]
    out_cnt = [0, 0, 0]

    # ---- input DMAs (SP queue) ----
    k = 0
    for si in range(len(SPLITS) - 1):
        for src, dst in ((xa, xsa), (xb, xsb)):
            nc.sync.dma_start(dst.ap()[:, :, SPLITS[si]:SPLITS[si + 1]],
                              src.ap()[:, :, SPLITS[si]:SPLITS[si + 1]]
                              ).then_inc(s_in[k], 16)
            k += 1

    # ---- ACT: preload the exp table during the input DMA ----
    const0 = nc.const_aps.aps[(fp32, 0.0)]
    nc.scalar.activation(dummy.ap()[:, :], const0, Exp, scale=1.0)

    # ---- PE warmup: ramp the clock while input streams (results unused) --
    for _ in range(6):
        nc.tensor.matmul(accs[2].ap()[:, 0:512], xsa.ap()[:, :, 0:128],
                         xsa.ap()[:, :, 0:512], start=True, stop=True,
                         perf_mode=DR)

    state = {"lvl": -1, "stops": 0, "stops_at": {}}
    xss = (xsa, xsb)

    def in_level(pi, end):
        """Index of the input DMA that must complete (issue order = xa/xb
        interleaved per split; per-engine FIFO makes dma_k done imply
        dma_j done for all j<k)."""
        for si in range(1, len(SPLITS)):
            if end <= SPLITS[si]:
                return 2 * (si - 1) + pi
        raise AssertionError(end)

    def emit_mm(ph, i, c0, s0, ww, pi, start, stop):
        base = i * 128
        m0 = base + c0 + s0
        lvl = max(in_level(pi, m0 + ww), in_level(pi, base + 128))
        if lvl > state["lvl"]:
            nc.tensor.wait_ge(s_in[lvl], 16)
            state["lvl"] = lvl
        xs = xss[pi]
        mm = nc.tensor.matmul(ph.ap()[:, s0:s0 + ww],
                              xs.ap()[:, :, base:base + 128],
                              xs.ap()[:, :, m0:m0 + ww],
                              start=start, stop=stop, perf_mode=DR)
        if stop:
            state["stops"] += 1
            mm.then_inc(s_mm, 1)

    SLICES = ((0, 1024, 0, 512), (0, 1024, 512, 512),
              (1024, 1024, 0, 512), (1024, 1024, 512, 512),
              (2048, 128, 0, 128))

    def acc_for(i, c0):
        """-> (psum handle, (sem, value) PE must wait before first write)."""
        if c0 == 2048:
            v = i
            if v >= 2:
                return naccs[v % 2], (s_act, 2 * (v - 2) + 2)
            return naccs[v % 2], None
        u = 2 * i + (c0 // 1024)
        if u >= 3:
            pu = u - 3
            if pu % 2 == 0:               # A-acc, consumed by ACT exp
                return accs[u % 3], (s_act, 2 * (pu // 2) + 1)
            return accs[u % 3], (s_dve, pu // 2 + 1)   # B-acc, DVE copy
        return accs[u % 3], None

    def emit_consumers(i):
        ot = ots[i % 3]
        slot = i % 3
        # ACT: exp slice A [0:1024]
        if i >= 3:
            nc.scalar.wait_ge(s_out[slot], 16 * out_cnt[slot])
        nc.scalar.wait_ge(s_mm, state["stops_at"][(i, 0)])
        nc.scalar.activation(ot.ap()[:, 0:1024],
                             accs[(2 * i) % 3].ap()[:, 0:1024], Exp,
                             scale=ESCALE).then_inc(s_act, 1)
        # DVE: raw copy slice B [1024:2048] (host exps these cols)
        if i >= 3:
            nc.vector.wait_ge(s_out[slot], 16 * out_cnt[slot])
        nc.vector.wait_ge(s_mm, state["stops_at"][(i, 1024)])
        nc.vector.tensor_copy(ot.ap()[:, 1024:2048],
                              accs[(2 * i + 1) % 3].ap()[:, 0:1024]
                              ).then_inc(s_dve, 1)
        # ACT: exp slice N [2048:2176]
        nc.scalar.wait_ge(s_mm, state["stops_at"][(i, 2048)])
        nc.scalar.activation(ot.ap()[:, 2048:W],
                             naccs[i % 2].ap()[:, 0:128], Exp,
                             scale=ESCALE).then_inc(s_act, 1)

    def emit_out(i):
        r0 = i * 128
        slot = i % 3
        if i == ROWS - 1:
            # split the final store so each piece starts as its producer ends
            nc.sync.wait_ge(s_act, 2 * i + 1)
            nc.sync.dma_start(out.ap()[r0:r0 + 128, 0:1024],
                              ots[slot].ap()[:, 0:1024]
                              ).then_inc(s_out[slot], 16)
            out_cnt[slot] += 1
            nc.sync.wait_ge(s_dve, i + 1)
            nc.sync.dma_start(out.ap()[r0:r0 + 128, 1024:2048],
                              ots[slot].ap()[:, 1024:2048]
                              ).then_inc(s_out[slot], 16)
            out_cnt[slot] += 1
            nc.sync.wait_ge(s_act, 2 * (i + 1))
            nc.sync.dma_start(out.ap()[r0:r0 + 128, 2048:W],
                              ots[slot].ap()[:, 2048:W]
                              ).then_inc(s_out[slot], 16)
            out_cnt[slot] += 1
        else:
            nc.sync.wait_ge(s_act, 2 * (i + 1))
            nc.sync.wait_ge(s_dve, i + 1)
            nc.sync.dma_start(out.ap()[r0:r0 + 128, :],
                              ots[slot].ap()[:, :]).then_inc(s_out[slot], 16)
            out_cnt[slot] += 1

    # ---- row 0 prologue: all pair-0 matmuls first (xb still in flight) ---
    for c0, wdt, s0, ww in SLICES:
        ph, _ = acc_for(0, c0)
        emit_mm(ph, 0, c0, s0, ww, 0, start=True, stop=False)
    for c0, wdt, s0, ww in SLICES:
        ph, _ = acc_for(0, c0)
        emit_mm(ph, 0, c0, s0, ww, 1, start=False, stop=True)
        state["stops_at"][(0, c0)] = state["stops"]
    emit_consumers(0)
    emit_out(0)

    # ---- steady state ----
    for i in range(1, ROWS):
        waited = set()
        for c0, wdt, s0, ww in SLICES:
            ph, wv = acc_for(i, c0)
            if wv is not None and (i, c0) not in waited:
                sem, val = wv
                nc.tensor.wait_ge(sem, val)
                waited.add((i, c0))
            for pi in range(2):
                emit_mm(ph, i, c0, s0, ww, pi, start=(pi == 0), stop=(pi == 1))
            state["stops_at"][(i, c0)] = state["stops"]
        emit_consumers(i)
        emit_out(i)

    for j in range(3):
        nc.sync.wait_ge(s_out[j], 16 * out_cnt[j])
    nc.compile()
    return nc


def _in_maps(x):
    x = np.asarray(x, dtype=np.float32)
    norm = np.sqrt(np.sum(x * x, axis=-1, keepdims=True))
    xn = x / np.maximum(norm, EPS)
    maps = []
    for c in range(N_CORES):
        b, h = c // 2, c % 2
        xb = xn[b]
        if h:
            xb = np.concatenate([xb[N // 2:], xb[:N // 2]], axis=0)
        # [N, D] -> xnT [D, N] -> [4 k-chunks, 128, N] -> [128, 4, N] fp8
        q = (xb.T * FP8_SCALE).astype(ml_dtypes.float8_e4m3)
        q = q.reshape(4, 128, N).transpose(1, 0, 2)
        maps.append({"xa": np.ascontiguousarray(q[:, 0:2]),
                     "xb": np.ascontiguousarray(q[:, 2:4])})
    return maps


def _assemble(results, out):
    for c in range(N_CORES):
        b, h = c // 2, c % 2
        o = out[b]
        strips = results[c]["out"].astype(np.float32)   # [2048, 2176]
        # device shipped raw dot sums for cols [1024:2048] (DVE path)
        strips[:, 1024:2048] = np.exp(ESCALE * strips[:, 1024:2048])
        for i in range(ROWS):
            r = i + ROWS * h
            s = strips[i * 128:(i + 1) * 128]
            e = min(T, NB - r)          # block-cols before wraparound
            o[r * 128:(r + 1) * 128, r * 128:r * 128 + e * 128] = s[:, :e * 128]
            if e > 1:                    # mirrors, skipping the diagonal t=0
                o[(r + 1) * 128:(r + e) * 128, r * 128:(r + 1) * 128] = \
                    s[:, 128:e * 128].T
            if e < T:                    # wrapped tail
                o[r * 128:(r + 1) * 128, 0:(T - e) * 128] = s[:, e * 128:]
                o[0:(T - e) * 128, r * 128:(r + 1) * 128] = s[:, e * 128:].T
    return out


def kernel(x: np.ndarray) -> np.ndarray:
    from concourse.bass_utils import run_bass_kernel_spmd

    x = np.asarray(x, dtype=np.float32)
    assert x.shape == (B, N, D)

    if "nc" not in _compiled:
        _compiled["nc"] = _build()
    nc = _compiled["nc"]

    res = run_bass_kernel_spmd(nc, _in_maps(x), list(range(N_CORES)))
    out = np.empty((B, N, N), dtype=np.float32)
    return _assemble([res.results[c] for c in range(N_CORES)], out)
